# revision 5
# baseline (speedup 1.0000x reference)
"""Trainium2 Bass kernel: full (non-causal) softmax attention.

Input:  query/key/value [1, 4096, 16, 128] f32 (B, S, H, D).
Output: [1, 4096, 16, 128] f32 = softmax(Q K^T / sqrt(D)) V per head.

Sharding: 16 heads over 8 cores -> 2 heads per core, no collectives.
Host pre-transposes Q,K per head to [D, S] (fp32) and V to bf16; the
device returns the UN-normalized attention output transposed [D, S] plus
a per-(head,qc) key-partial denominator tile [128, QC]; the host does
the final 128-way key-partition sum and the divide.

Device pipeline per head, per query-chunk QC (1024 queries):
  kt loop over 32 key chunks in period-3 groups (2 kt -> stA, 1 kt -> stB):
    stA [128,2048] fp32 psum (4 banks): scores for 2 key-chunks
       -> ONE ACT exp FD=2048 -> ptA bf16        (amortizes ACT's 352-cyc
    stB [128,1024] fp32 psum (2 banks): 1 chunk   per-call overhead; psum
       -> ACT exp FD=1024 -> ptB bf16            budget: 4+2+2 banks)
    OUT += V_kt^T @ pt[kt]   (bf16 moving, fp32r-free psum accumulation)
  den: binary tree on DVE: 16 pair-adds + 8 quad-adds in bf16 (2x mode),
       then 4+2+1 adds in fp32; the [128, QC] fp32 total DMAs to host.
ACT exp is the bottleneck at ~1049ns/kt-iter; PE ~965; DVE ~825.
"""

import os
import sys
from contextlib import ExitStack

import numpy as np

sys.path.insert(0, "/opt/trn_rl_repo")

import ml_dtypes
import concourse.bacc as bacc
import concourse.bass as bass
import concourse.tile as tile
from concourse import mybir
from concourse.bass_utils import run_bass_kernel_spmd

N_CORES = 8
S = 4096
H = 16
D = 128
HEADS_PER_CORE = H // N_CORES  # 2
KT_CHUNK = 128                  # keys per score tile (psum partition dim)
QC = 1024                       # queries per super-chunk
NMM = 512                       # moving free dim per matmul (psum bank fp32)
SCALE = float(D) ** -0.5

F32 = mybir.dt.float32
F32R = mybir.dt.float32r
BF16 = mybir.dt.bfloat16
ADD = mybir.AluOpType.add
EXP = mybir.ActivationFunctionType.Exp


def build_program(s=S, heads=HEADS_PER_CORE):
    nc = bacc.Bacc("TRN2", target_bir_lowering=False, debug=False,
                   num_devices=N_CORES)

    n_kt = s // KT_CHUNK
    n_qc = s // QC

    qt_d = nc.dram_tensor("qt", [heads, D, s], F32, kind="ExternalInput")
    kt_d = nc.dram_tensor("kt", [heads, D, s], F32, kind="ExternalInput")
    v_d = nc.dram_tensor("v", [heads, s, D], BF16, kind="ExternalInput")
    out_d = nc.dram_tensor("out", [heads, D, s], F32, kind="ExternalOutput")
    dent_d = nc.dram_tensor("dent", [heads, n_qc, 128, QC], F32,
                            kind="ExternalOutput")

    with tile.TileContext(nc) as tc, ExitStack() as ctx:
        qkv_pool = ctx.enter_context(tc.tile_pool(name="qkv", bufs=2))
        ptA_pool = ctx.enter_context(tc.tile_pool(name="ptA", bufs=3))
        ptB_pool = ctx.enter_context(tc.tile_pool(name="ptB", bufs=3))
        pair_pool = ctx.enter_context(tc.tile_pool(name="pair", bufs=4))
        quad_pool = ctx.enter_context(tc.tile_pool(name="quad", bufs=4))
        oct_pool = ctx.enter_context(tc.tile_pool(name="oct", bufs=4))
        hex_pool = ctx.enter_context(tc.tile_pool(name="hex", bufs=3))
        tot_pool = ctx.enter_context(tc.tile_pool(name="tot", bufs=2))
        osb_pool = ctx.enter_context(tc.tile_pool(name="osb", bufs=2))
        stA_pool = ctx.enter_context(
            tc.tile_pool(name="stA", bufs=1, space="PSUM"))
        stB_pool = ctx.enter_context(
            tc.tile_pool(name="stB", bufs=1, space="PSUM"))
        outp_pool = ctx.enter_context(
            tc.tile_pool(name="outp", bufs=1, space="PSUM"))

        def load_head(h):
            qt_sb = qkv_pool.tile([D, s], F32R, tag="qt")
            nc.sync.dma_start(out=qt_sb[:], in_=qt_d[h].bitcast(F32R))
            kt_sb = qkv_pool.tile([D, s], F32R, tag="kt")
            nc.sync.dma_start(out=kt_sb[:], in_=kt_d[h].bitcast(F32R))
            v_sb = qkv_pool.tile([128, n_kt, D], BF16, tag="v")
            nc.sync.dma_start(
                out=v_sb[:],
                in_=v_d[h].rearrange("(c p) d -> p c d", p=128))
            return qt_sb, kt_sb, v_sb

        heads_sb = [load_head(0)]
        pending = []  # deferred epilogue closures, drained 1/kt-iteration

        for h in range(heads):
            qt_sb, kt_sb, v_sb = heads_sb[h]
            if h + 1 < heads:
                heads_sb.append(load_head(h + 1))
            for qc in range(n_qc):
                q0 = qc * QC
                out_ps = outp_pool.tile([D, QC], F32, tag="outp")
                # den reduction ladder state
                pendB = []    # ptB APs waiting to be paired
                pairs = []    # bf16 [128,QC] (each = 2 kt) waiting
                quads = []    # bf16 (4 kt)
                octs = []     # f32 (8 kt)
                hexs = []     # f32 (16 kt)

                def den_push_pair(pr):
                    pairs.append(pr)
                    if len(pairs) == 2:
                        qd = quad_pool.tile([128, QC], BF16, tag="quad")
                        nc.vector.tensor_tensor(
                            qd[:], pairs[0], pairs[1], ADD)
                        pairs.clear()
                        quads.append(qd)
                    if len(quads) == 2:
                        oc = oct_pool.tile([128, QC], F32, tag="oct")
                        nc.vector.tensor_tensor(
                            oc[:], quads[0][:], quads[1][:], ADD)
                        quads.clear()
                        octs.append(oc)
                    if len(octs) == 2:
                        hx = hex_pool.tile([128, QC], F32, tag="hex")
                        nc.vector.tensor_tensor(
                            hx[:], octs[0][:], octs[1][:], ADD)
                        octs.clear()
                        hexs.append(hx)

                # kt groups: (2 kt -> stA, 1 kt -> stB) x10, then 2 kt -> stA
                groups = [('A', (3 * p, 3 * p + 1)) for p in range(10)]
                tail_b = [('B', (3 * p + 2,)) for p in range(10)]
                seq = []
                for a, b in zip(groups, tail_b):
                    seq += [a, b]
                seq.append(('A', (30, 31)))

                for kind, kts in seq:
                    if kind == 'A':
                        st = stA_pool.tile([128, 2 * QC], F32, tag="stA")
                        pt = ptA_pool.tile([128, 2 * QC], BF16, tag="ptA")
                    else:
                        st = stB_pool.tile([128, QC], F32, tag="stB")
                        pt = ptB_pool.tile([128, QC], BF16, tag="ptB")
                    for i, kt in enumerate(kts):
                        k0 = kt * KT_CHUNK
                        lhs_k = kt_sb[:, k0:k0 + KT_CHUNK]
                        for j in range(QC // NMM):
                            c0 = i * QC + j * NMM
                            nc.tensor.matmul(
                                st[:, c0:c0 + NMM],
                                lhs_k,
                                qt_sb[:, q0 + j * NMM:q0 + (j + 1) * NMM],
                                start=True, stop=True)
                    nc.scalar.activation(pt[:], st[:], EXP, scale=SCALE)
                    for i, kt in enumerate(kts):
                        lhs_v = v_sb[:, kt, :]
                        for j in range(QC // NMM):
                            nc.tensor.matmul(
                                out_ps[:, j * NMM:(j + 1) * NMM],
                                lhs_v,
                                pt[:, i * QC + j * NMM:i * QC + j * NMM + NMM],
                                start=(kt == 0), stop=(kt == n_kt - 1))
                    # den ladder input
                    if kind == 'A':
                        pr = pair_pool.tile([128, QC], BF16, tag="pair")
                        nc.vector.tensor_tensor(
                            pr[:], pt[:, 0:QC], pt[:, QC:2 * QC], ADD)
                        den_push_pair(pr[:])
                    else:
                        pendB.append(pt[:])
                        if len(pendB) == 2:
                            pr = pair_pool.tile([128, QC], BF16, tag="pair")
                            nc.vector.tensor_tensor(
                                pr[:], pendB[0], pendB[1], ADD)
                            pendB.clear()
                            den_push_pair(pr[:])
                    if pending:
                        pending.pop(0)()

                assert not (pendB or pairs or quads or octs) and len(hexs) == 2

                def finish(out_ps=out_ps, hexs=list(hexs), h=h, qc=qc, q0=q0):
                    def s1():
                        tot = tot_pool.tile([128, QC], F32, tag="tot")
                        nc.vector.tensor_tensor(
                            tot[:], hexs[0][:], hexs[1][:], ADD)
                        nc.sync.dma_start(out=dent_d[h, qc], in_=tot[:])

                    def s2():
                        out_sb = osb_pool.tile([D, QC], F32, tag="out_sb")
                        nc.vector.tensor_copy(out_sb[:], out_ps[:])
                        nc.sync.dma_start(
                            out=out_d[h][:, q0:q0 + QC], in_=out_sb[:])

                    return [s1, s2]

                pending.extend(finish())
        while pending:
            pending.pop(0)()

    nc.compile()
    return nc


def _install_ntff_hook():
    """Provide antenv.axon_hooks (absent in this image) so that
    run_bass_kernel_spmd(trace=True) can capture NTFF profiles via the
    axon .so."""
    try:
        from antenv.axon_hooks import get_axon_ntff_profile_hook  # noqa: F401
        return
    except ImportError:
        pass
    import contextlib
    import ctypes
    import types

    so_path = "/opt/axon/libaxon_pjrt.so"
    lib = ctypes.CDLL(so_path)
    if not hasattr(lib, "axon_start_nrt_profile"):
        return
    lib.axon_start_nrt_profile.argtypes = [
        ctypes.POINTER(ctypes.c_int64), ctypes.c_size_t]
    lib.axon_start_nrt_profile.restype = ctypes.c_int64
    lib.axon_stop_nrt_profile.argtypes = [ctypes.c_char_p]
    lib.axon_stop_nrt_profile.restype = ctypes.c_int64

    @contextlib.contextmanager
    def _hook(output_dir, device_ids):
        import jax
        jax.devices()
        if device_ids:
            ids = (ctypes.c_int64 * len(device_ids))(*device_ids)
            rc = lib.axon_start_nrt_profile(ids, len(device_ids))
        else:
            rc = lib.axon_start_nrt_profile(None, 0)
        if rc != 0:
            raise RuntimeError(f"axon_start_nrt_profile rc={rc}")
        try:
            yield
        finally:
            n = lib.axon_stop_nrt_profile(str(output_dir).encode())
            print(f"ntff profile: {n} file(s) written to {output_dir}")

    mod = types.ModuleType("antenv.axon_hooks")
    mod.get_axon_ntff_profile_hook = lambda: _hook
    mod.set_axon_ntff_profile_hook = lambda h: None
    import antenv
    sys.modules["antenv.axon_hooks"] = mod
    antenv.axon_hooks = mod


_CACHE = {}


def _get_program():
    key = "main"
    if key not in _CACHE:
        _CACHE[key] = build_program()
    return _CACHE[key]


def kernel(query, key, value, trace=False, **trace_kwargs):
    assert query.shape == (1, S, H, D)
    nc = _get_program()

    q = np.asarray(query, dtype=np.float32)[0]   # [S, H, D]
    k = np.asarray(key, dtype=np.float32)[0]
    v = np.asarray(value, dtype=np.float32)[0]

    in_maps = []
    for c in range(N_CORES):
        hs = slice(c * HEADS_PER_CORE, (c + 1) * HEADS_PER_CORE)
        # [S, h, D] -> [h, D, S]
        qt = np.ascontiguousarray(q[:, hs, :].transpose(1, 2, 0))
        kt = np.ascontiguousarray(k[:, hs, :].transpose(1, 2, 0))
        vv = np.ascontiguousarray(
            v[:, hs, :].transpose(1, 0, 2)).astype(ml_dtypes.bfloat16)
        in_maps.append({"qt": qt, "kt": kt, "v": vv})

    if trace:
        _install_ntff_hook()
    res = run_bass_kernel_spmd(nc, in_maps, core_ids=list(range(N_CORES)),
                               trace=trace, **trace_kwargs)

    out = np.empty((1, S, H, D), dtype=np.float32)
    n_qc = S // QC
    for c in range(N_CORES):
        o = res.results[c]["out"]      # [h, D, S] unnormalized
        dent = res.results[c]["dent"]  # [h, n_qc, 128, QC] key-partials
        den = dent.sum(axis=2).reshape(HEADS_PER_CORE, S)
        for i in range(HEADS_PER_CORE):
            out[0, :, c * HEADS_PER_CORE + i, :] = (o[i] / den[i][None, :]).T
    if trace:
        kernel.last_results = res
    return out


# revision 6
# speedup vs baseline: 1.0645x; 1.0645x over previous
"""Trainium2 Bass kernel: full (non-causal) softmax attention.

Input:  query/key/value [1, 4096, 16, 128] f32 (B, S, H, D).
Output: [1, 4096, 16, 128] f32 = softmax(Q K^T / sqrt(D)) V per head.

Sharding: 16 heads over 8 cores -> 2 heads per core, no collectives.
Host pre-transposes Q,K per head to [D, S] (fp32) and V to bf16; the
device returns the UN-normalized attention output transposed [D, S] plus
a per-(head,qc) key-partial denominator tile [128, QC]; the host does
the final 128-way key-partition sum and the divide.

Device pipeline per head, per query-chunk QC (1024 queries):
  kt loop over 32 key chunks in period-3 groups (2 kt -> stA, 1 kt -> stB):
    stA [128,2048] fp32 psum (4 banks): scores for 2 key-chunks
       -> ONE ACT exp FD=2048 -> ptA bf16        (amortizes ACT's 352-cyc
    stB [128,1024] fp32 psum (2 banks): 1 chunk   per-call overhead; psum
       -> ACT exp FD=1024 -> ptB bf16            budget: 4+2+2 banks)
    OUT += V_kt^T @ pt[kt]   (bf16 moving, fp32r-free psum accumulation)
  den: binary tree on DVE: 16 pair-adds + 8 quad-adds in bf16 (2x mode),
       then 4+2+1 adds in fp32; the [128, QC] fp32 total DMAs to host.
ACT exp is the bottleneck at ~1049ns/kt-iter; PE ~965; DVE ~825.
"""

import os
import sys
from contextlib import ExitStack

import numpy as np

sys.path.insert(0, "/opt/trn_rl_repo")

import ml_dtypes
import concourse.bacc as bacc
import concourse.bass as bass
import concourse.tile as tile
from concourse import mybir
from concourse.bass_utils import run_bass_kernel_spmd

N_CORES = 8
S = 4096
H = 16
D = 128
HEADS_PER_CORE = H // N_CORES  # 2
KT_CHUNK = 128                  # keys per score tile (psum partition dim)
QC = 1024                       # queries per super-chunk
NMM = 512                       # moving free dim per matmul (psum bank fp32)
SCALE = float(D) ** -0.5

F32 = mybir.dt.float32
F32R = mybir.dt.float32r
BF16 = mybir.dt.bfloat16
ADD = mybir.AluOpType.add
EXP = mybir.ActivationFunctionType.Exp


def build_program(s=S, heads=HEADS_PER_CORE):
    nc = bacc.Bacc("TRN2", target_bir_lowering=False, debug=False,
                   num_devices=N_CORES)

    n_kt = s // KT_CHUNK
    n_qc = s // QC

    qt_d = nc.dram_tensor("qt", [heads, D, s], F32, kind="ExternalInput")
    kt_d = nc.dram_tensor("kt", [heads, D, s], F32, kind="ExternalInput")
    v_d = nc.dram_tensor("v", [heads, s, D], BF16, kind="ExternalInput")
    out_d = nc.dram_tensor("out", [heads, D, s], F32, kind="ExternalOutput")
    dent_d = nc.dram_tensor("dent", [heads, n_qc, 128, QC], F32,
                            kind="ExternalOutput")

    with tile.TileContext(nc) as tc, ExitStack() as ctx:
        qkv_pool = ctx.enter_context(tc.tile_pool(name="qkv", bufs=2))
        ptA_pool = ctx.enter_context(tc.tile_pool(name="ptA", bufs=3))
        ptB_pool = ctx.enter_context(tc.tile_pool(name="ptB", bufs=3))
        pair_pool = ctx.enter_context(tc.tile_pool(name="pair", bufs=4))
        quad_pool = ctx.enter_context(tc.tile_pool(name="quad", bufs=4))
        oct_pool = ctx.enter_context(tc.tile_pool(name="oct", bufs=4))
        hex_pool = ctx.enter_context(tc.tile_pool(name="hex", bufs=3))
        tot_pool = ctx.enter_context(tc.tile_pool(name="tot", bufs=2))
        osb_pool = ctx.enter_context(tc.tile_pool(name="osb", bufs=2))
        stA_pool = ctx.enter_context(
            tc.tile_pool(name="stA", bufs=1, space="PSUM"))
        stB_pool = ctx.enter_context(
            tc.tile_pool(name="stB", bufs=1, space="PSUM"))
        outp_pool = ctx.enter_context(
            tc.tile_pool(name="outp", bufs=1, space="PSUM"))

        def load_head(h):
            qt_sb = qkv_pool.tile([D, s], F32R, tag="qt")
            nc.sync.dma_start(out=qt_sb[:], in_=qt_d[h].bitcast(F32R))
            kt_sb = qkv_pool.tile([D, s], F32R, tag="kt")
            nc.sync.dma_start(out=kt_sb[:], in_=kt_d[h].bitcast(F32R))
            v_sb = qkv_pool.tile([128, n_kt, D], BF16, tag="v")
            nc.sync.dma_start(
                out=v_sb[:],
                in_=v_d[h].rearrange("(c p) d -> p c d", p=128))
            return qt_sb, kt_sb, v_sb

        heads_sb = [load_head(0)]
        pending = []  # deferred epilogue closures, drained 1/kt-iteration

        for h in range(heads):
            qt_sb, kt_sb, v_sb = heads_sb[h]
            if h + 1 < heads:
                heads_sb.append(load_head(h + 1))
            for qc in range(n_qc):
                q0 = qc * QC
                out_ps = outp_pool.tile([D, QC], F32, tag="outp")
                # den reduction ladder state
                pendB = []    # ptB APs waiting to be paired
                pairs = []    # bf16 [128,QC] (each = 2 kt) waiting
                quads = []    # bf16 (4 kt)
                octs = []     # f32 (8 kt)
                hexs = []     # f32 (16 kt)

                def den_push_pair(pr):
                    pairs.append(pr)
                    if len(pairs) == 2:
                        qd = quad_pool.tile([128, QC], BF16, tag="quad")
                        nc.vector.tensor_tensor(
                            qd[:], pairs[0], pairs[1], ADD)
                        pairs.clear()
                        quads.append(qd)
                    if len(quads) == 2:
                        oc = oct_pool.tile([128, QC], F32, tag="oct")
                        nc.vector.tensor_tensor(
                            oc[:], quads[0][:], quads[1][:], ADD)
                        quads.clear()
                        octs.append(oc)
                    if len(octs) == 2:
                        hx = hex_pool.tile([128, QC], F32, tag="hex")
                        nc.vector.tensor_tensor(
                            hx[:], octs[0][:], octs[1][:], ADD)
                        octs.clear()
                        hexs.append(hx)

                # kt groups: (2 kt -> stA, 1 kt -> stB) x10, then 2 kt -> stA
                groups = [('A', (3 * p, 3 * p + 1)) for p in range(10)]
                tail_b = [('B', (3 * p + 2,)) for p in range(10)]
                seq = []
                for a, b in zip(groups, tail_b):
                    seq += [a, b]
                seq.append(('A', (30, 31)))

                def emit_pv_den(kind, kts, pt):
                    # PV matmuls + den-ladder input for a finished group
                    for i, kt in enumerate(kts):
                        lhs_v = v_sb[:, kt, :]
                        for j in range(QC // NMM):
                            nc.tensor.matmul(
                                out_ps[:, j * NMM:(j + 1) * NMM],
                                lhs_v,
                                pt[:, i * QC + j * NMM:i * QC + j * NMM + NMM],
                                start=(kt == 0), stop=(kt == n_kt - 1))
                    if kind == 'A':
                        pr = pair_pool.tile([128, QC], BF16, tag="pair")
                        nc.vector.tensor_tensor(
                            pr[:], pt[:, 0:QC], pt[:, QC:2 * QC], ADD)
                        den_push_pair(pr[:])
                    else:
                        pendB.append(pt[:])
                        if len(pendB) == 2:
                            pr = pair_pool.tile([128, QC], BF16, tag="pair")
                            nc.vector.tensor_tensor(
                                pr[:], pendB[0], pendB[1], ADD)
                            pendB.clear()
                            den_push_pair(pr[:])

                # Software-pipelined emission: the PE queue is strict FIFO,
                # so PV (which waits on exp) must trail by one group or it
                # blocks the next group's score matmuls in the queue.
                prev = None
                for kind, kts in seq:
                    if kind == 'A':
                        st = stA_pool.tile([128, 2 * QC], F32, tag="stA")
                        pt = ptA_pool.tile([128, 2 * QC], BF16, tag="ptA")
                    else:
                        st = stB_pool.tile([128, QC], F32, tag="stB")
                        pt = ptB_pool.tile([128, QC], BF16, tag="ptB")
                    for i, kt in enumerate(kts):
                        k0 = kt * KT_CHUNK
                        lhs_k = kt_sb[:, k0:k0 + KT_CHUNK]
                        for j in range(QC // NMM):
                            c0 = i * QC + j * NMM
                            nc.tensor.matmul(
                                st[:, c0:c0 + NMM],
                                lhs_k,
                                qt_sb[:, q0 + j * NMM:q0 + (j + 1) * NMM],
                                start=True, stop=True)
                    if prev is not None:
                        emit_pv_den(*prev)
                    nc.scalar.activation(pt[:], st[:], EXP, scale=SCALE)
                    prev = (kind, kts, pt)
                    if pending:
                        pending.pop(0)()
                emit_pv_den(*prev)

                assert not (pendB or pairs or quads or octs) and len(hexs) == 2

                def finish(out_ps=out_ps, hexs=list(hexs), h=h, qc=qc, q0=q0):
                    def s1():
                        tot = tot_pool.tile([128, QC], F32, tag="tot")
                        nc.vector.tensor_tensor(
                            tot[:], hexs[0][:], hexs[1][:], ADD)
                        nc.sync.dma_start(out=dent_d[h, qc], in_=tot[:])

                    def s2():
                        out_sb = osb_pool.tile([D, QC], F32, tag="out_sb")
                        nc.vector.tensor_copy(out_sb[:], out_ps[:])
                        nc.sync.dma_start(
                            out=out_d[h][:, q0:q0 + QC], in_=out_sb[:])

                    return [s1, s2]

                pending.extend(finish())
        while pending:
            pending.pop(0)()

    nc.compile()
    return nc


def _install_ntff_hook():
    """Provide antenv.axon_hooks (absent in this image) so that
    run_bass_kernel_spmd(trace=True) can capture NTFF profiles via the
    axon .so."""
    try:
        from antenv.axon_hooks import get_axon_ntff_profile_hook  # noqa: F401
        return
    except ImportError:
        pass
    import contextlib
    import ctypes
    import types

    so_path = "/opt/axon/libaxon_pjrt.so"
    lib = ctypes.CDLL(so_path)
    if not hasattr(lib, "axon_start_nrt_profile"):
        return
    lib.axon_start_nrt_profile.argtypes = [
        ctypes.POINTER(ctypes.c_int64), ctypes.c_size_t]
    lib.axon_start_nrt_profile.restype = ctypes.c_int64
    lib.axon_stop_nrt_profile.argtypes = [ctypes.c_char_p]
    lib.axon_stop_nrt_profile.restype = ctypes.c_int64

    @contextlib.contextmanager
    def _hook(output_dir, device_ids):
        import jax
        jax.devices()
        if device_ids:
            ids = (ctypes.c_int64 * len(device_ids))(*device_ids)
            rc = lib.axon_start_nrt_profile(ids, len(device_ids))
        else:
            rc = lib.axon_start_nrt_profile(None, 0)
        if rc != 0:
            raise RuntimeError(f"axon_start_nrt_profile rc={rc}")
        try:
            yield
        finally:
            n = lib.axon_stop_nrt_profile(str(output_dir).encode())
            print(f"ntff profile: {n} file(s) written to {output_dir}")

    mod = types.ModuleType("antenv.axon_hooks")
    mod.get_axon_ntff_profile_hook = lambda: _hook
    mod.set_axon_ntff_profile_hook = lambda h: None
    import antenv
    sys.modules["antenv.axon_hooks"] = mod
    antenv.axon_hooks = mod


_CACHE = {}


def _get_program():
    key = "main"
    if key not in _CACHE:
        _CACHE[key] = build_program()
    return _CACHE[key]


def kernel(query, key, value, trace=False, **trace_kwargs):
    assert query.shape == (1, S, H, D)
    nc = _get_program()

    q = np.asarray(query, dtype=np.float32)[0]   # [S, H, D]
    k = np.asarray(key, dtype=np.float32)[0]
    v = np.asarray(value, dtype=np.float32)[0]

    in_maps = []
    for c in range(N_CORES):
        hs = slice(c * HEADS_PER_CORE, (c + 1) * HEADS_PER_CORE)
        # [S, h, D] -> [h, D, S]
        qt = np.ascontiguousarray(q[:, hs, :].transpose(1, 2, 0))
        kt = np.ascontiguousarray(k[:, hs, :].transpose(1, 2, 0))
        vv = np.ascontiguousarray(
            v[:, hs, :].transpose(1, 0, 2)).astype(ml_dtypes.bfloat16)
        in_maps.append({"qt": qt, "kt": kt, "v": vv})

    if trace:
        _install_ntff_hook()
    res = run_bass_kernel_spmd(nc, in_maps, core_ids=list(range(N_CORES)),
                               trace=trace, **trace_kwargs)

    out = np.empty((1, S, H, D), dtype=np.float32)
    n_qc = S // QC
    for c in range(N_CORES):
        o = res.results[c]["out"]      # [h, D, S] unnormalized
        dent = res.results[c]["dent"]  # [h, n_qc, 128, QC] key-partials
        den = dent.sum(axis=2).reshape(HEADS_PER_CORE, S)
        for i in range(HEADS_PER_CORE):
            out[0, :, c * HEADS_PER_CORE + i, :] = (o[i] / den[i][None, :]).T
    if trace:
        kernel.last_results = res
    return out


# revision 8
# speedup vs baseline: 1.2959x; 1.2174x over previous
"""Trainium2 Bass kernel: full (non-causal) softmax attention.

Input:  query/key/value [1, 4096, 16, 128] f32 (B, S, H, D).
Output: [1, 4096, 16, 128] f32 = softmax(Q K^T / sqrt(D)) V per head.

Sharding: 16 heads over 8 cores -> 2 heads per core, no collectives.
Host pre-transposes Q,K per head to [D, S] (fp32) and V to bf16; the
device returns the UN-normalized attention output transposed [D, S] plus
a per-(head,qc) key-partial denominator tile [128, QC]; the host does
the final 128-way key-partition sum and the divide.

Device pipeline per head, per query-chunk QC (1024 queries):
  kt loop over 32 key chunks in period-3 groups (2 kt -> stA, 1 kt -> stB):
    stA [128,2048] fp32 psum (4 banks): scores for 2 key-chunks
       -> ONE ACT exp FD=2048 -> ptA bf16        (amortizes ACT's 352-cyc
    stB [128,1024] fp32 psum (2 banks): 1 chunk   per-call overhead; psum
       -> ACT exp FD=1024 -> ptB bf16            budget: 4+2+2 banks)
    OUT += V_kt^T @ pt[kt]   (bf16 moving, fp32r-free psum accumulation)
  den: binary tree on DVE: 16 pair-adds + 8 quad-adds in bf16 (2x mode),
       then 4+2+1 adds in fp32; the [128, QC] fp32 total DMAs to host.
ACT exp is the bottleneck at ~1049ns/kt-iter; PE ~965; DVE ~825.
"""

import os
import sys
from contextlib import ExitStack

import numpy as np

sys.path.insert(0, "/opt/trn_rl_repo")

import ml_dtypes
import concourse.bacc as bacc
import concourse.bass as bass
import concourse.tile as tile
from concourse import mybir
from concourse.bass_utils import run_bass_kernel_spmd

N_CORES = 8
S = 4096
H = 16
D = 128
HEADS_PER_CORE = H // N_CORES  # 2
KT_CHUNK = 128                  # keys per score tile (psum partition dim)
QC = 1024                       # queries per super-chunk
NMM = 512                       # moving free dim per matmul (psum bank fp32)
SCALE = float(D) ** -0.5

F32 = mybir.dt.float32
F32R = mybir.dt.float32r
BF16 = mybir.dt.bfloat16
ADD = mybir.AluOpType.add
EXP = mybir.ActivationFunctionType.Exp


def build_program(s=S, heads=HEADS_PER_CORE):
    nc = bacc.Bacc("TRN2", target_bir_lowering=False, debug=False,
                   num_devices=N_CORES)

    n_kt = s // KT_CHUNK
    n_qc = s // QC

    qt_d = nc.dram_tensor("qt", [heads, D, s], F32, kind="ExternalInput")
    kt_d = nc.dram_tensor("kt", [heads, D, s], F32, kind="ExternalInput")
    v_d = nc.dram_tensor("v", [heads, s, D], BF16, kind="ExternalInput")
    out_d = nc.dram_tensor("out", [heads, D, s], F32, kind="ExternalOutput")
    dent_d = nc.dram_tensor("dent", [heads, n_qc, 128, QC], F32,
                            kind="ExternalOutput")

    with tile.TileContext(nc) as tc, ExitStack() as ctx:
        qkv_pool = ctx.enter_context(tc.tile_pool(name="qkv", bufs=2))
        ptA_pool = ctx.enter_context(tc.tile_pool(name="ptA", bufs=3))
        ptB_pool = ctx.enter_context(tc.tile_pool(name="ptB", bufs=3))
        pair_pool = ctx.enter_context(tc.tile_pool(name="pair", bufs=4))
        quad_pool = ctx.enter_context(tc.tile_pool(name="quad", bufs=4))
        oct_pool = ctx.enter_context(tc.tile_pool(name="oct", bufs=4))
        hex_pool = ctx.enter_context(tc.tile_pool(name="hex", bufs=3))
        tot_pool = ctx.enter_context(tc.tile_pool(name="tot", bufs=2))
        osb_pool = ctx.enter_context(tc.tile_pool(name="osb", bufs=2))
        stA_pool = ctx.enter_context(
            tc.tile_pool(name="stA", bufs=1, space="PSUM"))
        stB_pool = ctx.enter_context(
            tc.tile_pool(name="stB", bufs=1, space="PSUM"))
        outp_pool = ctx.enter_context(
            tc.tile_pool(name="outp", bufs=1, space="PSUM"))

        def load_head(h):
            qt_sb = qkv_pool.tile([D, s], F32R, tag="qt")
            nc.sync.dma_start(out=qt_sb[:], in_=qt_d[h].bitcast(F32R))
            kt_sb = qkv_pool.tile([D, s], F32R, tag="kt")
            nc.sync.dma_start(out=kt_sb[:], in_=kt_d[h].bitcast(F32R))
            v_sb = qkv_pool.tile([128, n_kt, D], BF16, tag="v")
            nc.sync.dma_start(
                out=v_sb[:],
                in_=v_d[h].rearrange("(c p) d -> p c d", p=128))
            return qt_sb, kt_sb, v_sb

        heads_sb = [load_head(0)]
        pending = []  # deferred epilogue closures, drained 1/kt-iteration

        for h in range(heads):
            qt_sb, kt_sb, v_sb = heads_sb[h]
            if h + 1 < heads:
                heads_sb.append(load_head(h + 1))
            for qc in range(n_qc):
                q0 = qc * QC
                out_ps = outp_pool.tile([D, QC], F32, tag="outp")
                # den reduction ladder state
                pendB = []    # ptB APs waiting to be paired
                pairs = []    # bf16 [128,QC] (each = 2 kt) waiting
                quads = []    # bf16 (4 kt)
                octs = []     # f32 (8 kt)
                hexs = []     # f32 (16 kt)

                def den_push_pair(pr):
                    pairs.append(pr)
                    if len(pairs) == 2:
                        qd = quad_pool.tile([128, QC], BF16, tag="quad")
                        nc.vector.tensor_tensor(
                            qd[:], pairs[0], pairs[1], ADD)
                        pairs.clear()
                        quads.append(qd)
                    if len(quads) == 2:
                        oc = oct_pool.tile([128, QC], F32, tag="oct")
                        nc.vector.tensor_tensor(
                            oc[:], quads[0][:], quads[1][:], ADD)
                        quads.clear()
                        octs.append(oc)
                    if len(octs) == 2:
                        hx = hex_pool.tile([128, QC], F32, tag="hex")
                        nc.vector.tensor_tensor(
                            hx[:], octs[0][:], octs[1][:], ADD)
                        octs.clear()
                        hexs.append(hx)

                # kt groups: (2 kt -> stA, 1 kt -> stB) x10, then 2 kt -> stA
                groups = [('A', (3 * p, 3 * p + 1)) for p in range(10)]
                tail_b = [('B', (3 * p + 2,)) for p in range(10)]
                seq = []
                for a, b in zip(groups, tail_b):
                    seq += [a, b]
                seq.append(('A', (30, 31)))

                def emit_pv_den(kind, kts, pt):
                    # PV matmuls + den-ladder input for a finished group
                    for i, kt in enumerate(kts):
                        lhs_v = v_sb[:, kt, :]
                        for j in range(QC // NMM):
                            nc.tensor.matmul(
                                out_ps[:, j * NMM:(j + 1) * NMM],
                                lhs_v,
                                pt[:, i * QC + j * NMM:i * QC + j * NMM + NMM],
                                start=(kt == 0), stop=(kt == n_kt - 1))
                    if kind == 'A':
                        pr = pair_pool.tile([128, QC], BF16, tag="pair")
                        nc.vector.tensor_tensor(
                            pr[:], pt[:, 0:QC], pt[:, QC:2 * QC], ADD)
                        den_push_pair(pr[:])
                    else:
                        pendB.append(pt[:])
                        if len(pendB) == 2:
                            pr = pair_pool.tile([128, QC], BF16, tag="pair")
                            nc.vector.tensor_tensor(
                                pr[:], pendB[0], pendB[1], ADD)
                            pendB.clear()
                            den_push_pair(pr[:])

                # Software-pipelined emission: the PE queue is strict FIFO,
                # so PV (which waits on exp) must trail by TWO groups —
                # the PE needs >= one full FD=2048 exp (2us) of queued work
                # between a group's scores and its PV, else it stalls and
                # the stall cascades into ACT idle time.
                inflight = []
                for kind, kts in seq:
                    if kind == 'A':
                        st = stA_pool.tile([128, 2 * QC], F32, tag="stA")
                        pt = ptA_pool.tile([128, 2 * QC], BF16, tag="ptA")
                    else:
                        st = stB_pool.tile([128, QC], F32, tag="stB")
                        pt = ptB_pool.tile([128, QC], BF16, tag="ptB")
                    for i, kt in enumerate(kts):
                        k0 = kt * KT_CHUNK
                        lhs_k = kt_sb[:, k0:k0 + KT_CHUNK]
                        for j in range(QC // NMM):
                            c0 = i * QC + j * NMM
                            nc.tensor.matmul(
                                st[:, c0:c0 + NMM],
                                lhs_k,
                                qt_sb[:, q0 + j * NMM:q0 + (j + 1) * NMM],
                                start=True, stop=True)
                    if len(inflight) == 2:
                        emit_pv_den(*inflight.pop(0))
                    nc.scalar.activation(pt[:], st[:], EXP, scale=SCALE)
                    inflight.append((kind, kts, pt))
                    if pending:
                        pending.pop(0)()
                while inflight:
                    emit_pv_den(*inflight.pop(0))

                assert not (pendB or pairs or quads or octs) and len(hexs) == 2

                def finish(out_ps=out_ps, hexs=list(hexs), h=h, qc=qc, q0=q0):
                    def s1():
                        tot = tot_pool.tile([128, QC], F32, tag="tot")
                        nc.vector.tensor_tensor(
                            tot[:], hexs[0][:], hexs[1][:], ADD)
                        nc.sync.dma_start(out=dent_d[h, qc], in_=tot[:])

                    def s2():
                        out_sb = osb_pool.tile([D, QC], F32, tag="out_sb")
                        nc.vector.tensor_copy(out_sb[:], out_ps[:])
                        nc.sync.dma_start(
                            out=out_d[h][:, q0:q0 + QC], in_=out_sb[:])

                    return [s1, s2]

                pending.extend(finish())
        while pending:
            pending.pop(0)()

    nc.compile()
    return nc


def _install_ntff_hook():
    """Provide antenv.axon_hooks (absent in this image) so that
    run_bass_kernel_spmd(trace=True) can capture NTFF profiles via the
    axon .so."""
    try:
        from antenv.axon_hooks import get_axon_ntff_profile_hook  # noqa: F401
        return
    except ImportError:
        pass
    import contextlib
    import ctypes
    import types

    so_path = "/opt/axon/libaxon_pjrt.so"
    lib = ctypes.CDLL(so_path)
    if not hasattr(lib, "axon_start_nrt_profile"):
        return
    lib.axon_start_nrt_profile.argtypes = [
        ctypes.POINTER(ctypes.c_int64), ctypes.c_size_t]
    lib.axon_start_nrt_profile.restype = ctypes.c_int64
    lib.axon_stop_nrt_profile.argtypes = [ctypes.c_char_p]
    lib.axon_stop_nrt_profile.restype = ctypes.c_int64

    @contextlib.contextmanager
    def _hook(output_dir, device_ids):
        import jax
        jax.devices()
        if device_ids:
            ids = (ctypes.c_int64 * len(device_ids))(*device_ids)
            rc = lib.axon_start_nrt_profile(ids, len(device_ids))
        else:
            rc = lib.axon_start_nrt_profile(None, 0)
        if rc != 0:
            raise RuntimeError(f"axon_start_nrt_profile rc={rc}")
        try:
            yield
        finally:
            n = lib.axon_stop_nrt_profile(str(output_dir).encode())
            print(f"ntff profile: {n} file(s) written to {output_dir}")

    mod = types.ModuleType("antenv.axon_hooks")
    mod.get_axon_ntff_profile_hook = lambda: _hook
    mod.set_axon_ntff_profile_hook = lambda h: None
    import antenv
    sys.modules["antenv.axon_hooks"] = mod
    antenv.axon_hooks = mod


_CACHE = {}


def _get_program():
    key = "main"
    if key not in _CACHE:
        _CACHE[key] = build_program()
    return _CACHE[key]


def kernel(query, key, value, trace=False, **trace_kwargs):
    assert query.shape == (1, S, H, D)
    nc = _get_program()

    q = np.asarray(query, dtype=np.float32)[0]   # [S, H, D]
    k = np.asarray(key, dtype=np.float32)[0]
    v = np.asarray(value, dtype=np.float32)[0]

    in_maps = []
    for c in range(N_CORES):
        hs = slice(c * HEADS_PER_CORE, (c + 1) * HEADS_PER_CORE)
        # [S, h, D] -> [h, D, S]
        qt = np.ascontiguousarray(q[:, hs, :].transpose(1, 2, 0))
        kt = np.ascontiguousarray(k[:, hs, :].transpose(1, 2, 0))
        vv = np.ascontiguousarray(
            v[:, hs, :].transpose(1, 0, 2)).astype(ml_dtypes.bfloat16)
        in_maps.append({"qt": qt, "kt": kt, "v": vv})

    if trace:
        _install_ntff_hook()
    res = run_bass_kernel_spmd(nc, in_maps, core_ids=list(range(N_CORES)),
                               trace=trace, **trace_kwargs)

    out = np.empty((1, S, H, D), dtype=np.float32)
    n_qc = S // QC
    for c in range(N_CORES):
        o = res.results[c]["out"]      # [h, D, S] unnormalized
        dent = res.results[c]["dent"]  # [h, n_qc, 128, QC] key-partials
        den = dent.sum(axis=2).reshape(HEADS_PER_CORE, S)
        for i in range(HEADS_PER_CORE):
            out[0, :, c * HEADS_PER_CORE + i, :] = (o[i] / den[i][None, :]).T
    if trace:
        kernel.last_results = res
    return out
